# revision 6
# baseline (speedup 1.0000x reference)
"""CGNN message-passing kernel for Trainium2, 8 NeuronCores (v2).

Strategy (v2 — over the v1 baseline):
  - Same algebraic reduction: per-node message table m' = dinv*(alpha*xn +
    (1-alpha)*xa); AllGather replicates the bf16 table; edge phase gathers
    source rows per target tile and scatter-adds via one-hot matmuls.
  - v2 performance restructure, driven by the instruction cost model:
    * SWDGE gathers batched across groups of 3 target tiles (2 calls/group
      instead of ~3.5 calls/tile): the 994ns fixed descriptor-gen overhead
      per call drops from ~170us to ~34us of Pool time. Ring enlarged via
      dynamic_dma_scratch_size=65536.
    * All moving matmul data in bf16 (4x fewer PE cycles vs f32), all big
      elementwise ops on 2-byte packed operands for the DVE fast paths.
    * One-hot built target-major (s_oh[p, t*KT+k]) so the broadcast of the
      per-chunk target id lands on a middle AP dim, keeping last dims packed
      (DVE 2x/4x modes stay enabled); the aggregation matmul consumes
      strided chunk columns instead.
    * dinv column-scale table broadcast once via gpsimd partition_broadcast
      (Pool engine, idle during the node phase).
    * x loaded as bf16 in 2 bulk DMAs; per-slab m' written in one DMA;
      outputs staged per group.
  - Host work: index preprocessing (sort/bincount/layout), dinv=rsqrt(deg),
    dtype casts, and shard/unshard data movement. All FLOPs on device.
"""
import numpy as np
import ml_dtypes

BF16 = ml_dtypes.bfloat16
N_CORES = 8
P = 128
IN_DIM = 256
HID = 128
HALF = 64
OUT_DIM = 2
LRELU_SLOPE = 0.01
SLAB_TILES = 4          # node-phase tiles per slab (nn <= 512)
G_TILES = 3             # edge-phase target tiles per gather group


def _host_plan(x, edge_index):
    """Index preprocessing + data layout. Returns dict of np arrays + meta."""
    n = x.shape[0]
    NP = ((n + 1023) // 1024) * 1024          # divisible by 8*128
    nsh = NP // N_CORES                        # nodes per core
    t_c = nsh // P                             # target tiles per core
    ntiles = NP // P

    ei = np.asarray(edge_index)
    row = ei[0].astype(np.int64)
    col = ei[1].astype(np.int64)
    loops = np.arange(n, dtype=np.int64)
    row_a = np.concatenate([row, loops])
    col_a = np.concatenate([col, loops])

    deg = np.bincount(col_a, minlength=NP).astype(np.float32)
    deg[n:] = 1.0                              # pad nodes: keep m' finite
    dinv = 1.0 / np.sqrt(deg)

    order = np.argsort(col_a, kind="stable")
    rs = row_a[order].astype(np.int32)
    cs = col_a[order]

    # Table rows are stored slab-permuted: within each 512-node slab of a
    # core's shard, node (p, j) lives at row base + p*nt + j so the node
    # phase writes 1KB-contiguous per partition. Remap gather indices.
    SLAB = SLAB_TILES * P
    loc = np.arange(nsh, dtype=np.int64)
    blk = loc // SLAB
    within = loc % SLAB
    nt_b = np.minimum(SLAB_TILES, (nsh - blk * SLAB) // P)
    perm_loc = blk * SLAB + (within % P) * nt_b + within // P
    perm = np.concatenate([c * nsh + perm_loc for c in range(N_CORES)])
    rs = perm[rs].astype(np.int32)

    h0 = NP // 2
    assert h0 <= 32767, "table half exceeds int16 index range"
    # order edges by (tile, half) so each tile's lo-half edges precede hi-half
    half_e = (rs >= h0).astype(np.int64)
    key = (cs // P) * 2 + half_e
    order2 = np.argsort(key, kind="stable")
    rs = rs[order2]
    cs = cs[order2]
    key = key[order2]

    gb = np.searchsorted(key, np.arange(0, 2 * ntiles + 1))  # group bounds
    glo = gb[0:-1:2]
    ghi = gb[1::2]
    gend = gb[2::2]
    n_lo = ghi - glo                           # per tile lo-edge counts
    n_hi = gend - ghi
    kl_j = -(-n_lo // P)                       # lo chunks per tile
    kh_j = -(-n_hi // P)
    tile_order = np.lexsort((kh_j, kl_j))      # tiles sorted by (kl, kh)
    # assign[c, s] = global tile handled by core c at slot s
    assign = tile_order.reshape(t_c, N_CORES).T
    kl_a = kl_j[assign]                        # [cores, slots]
    kh_a = kh_j[assign]
    KL = np.maximum(1, kl_a.max(0))            # per-slot chunk counts (max
    KH = np.maximum(1, kh_a.max(0))            # over cores: SPMD-uniform)
    KT = int((KL + KH).max())                  # uniform per-slot cm width

    # per-tile local int16 indices + per-chunk colmod, padded to slot maxes
    KLmax, KHmax = int(KL.max()), int(KH.max())
    idx_lo = np.zeros((ntiles, P, KLmax), np.int16)
    idx_hi = np.zeros((ntiles, P, KHmax), np.int16)
    slot_of = np.empty(ntiles, np.int64)
    slot_of[tile_order] = np.arange(ntiles) // N_CORES
    cm_tile = np.full((ntiles, P, KT), 999.0, np.float32)

    m = len(cs)
    j_e = (cs // P).astype(np.int64)
    is_hi = rs >= h0
    epos = np.arange(m, dtype=np.int64)
    epos = np.where(is_hi, epos - ghi[j_e], epos - glo[j_e])
    c_e = epos // P
    p_e = epos % P
    lo_m = ~is_hi
    idx_lo[j_e[lo_m], p_e[lo_m], c_e[lo_m]] = rs[lo_m].astype(np.int16)
    idx_hi[j_e[is_hi], p_e[is_hi], c_e[is_hi]] = (rs[is_hi] - h0).astype(np.int16)
    cm_e = (cs - j_e * P).astype(np.float32)
    # cm column: lo chunks [0, KL[slot]), hi chunks [KL[slot], +kh)
    # (KL is the slot max — must match the device's matmul indexing)
    c_cm = np.where(is_hi, KL[slot_of[j_e]] + c_e, c_e)
    cm_tile[j_e, p_e, c_cm] = cm_e

    # wrap into ucode layout: [P, K*8] int16 (16-row wrap, replicated x8)
    def wrap_rep(arr):      # [T, P, K] -> [T, P, K*8]
        t, _, k = arr.shape
        flat = arr.transpose(0, 2, 1).reshape(t, k * P)        # chunk-major
        blk = flat.reshape(t, k * 8, 16).transpose(0, 2, 1)    # [t, 16, k*8]
        return np.ascontiguousarray(
            np.repeat(blk, 8, axis=0).reshape(t, 8 * 16, k * 8))

    # group slots
    n_g = -(-t_c // G_TILES)
    g_slots = [list(range(g * G_TILES, min((g + 1) * G_TILES, t_c)))
               for g in range(n_g)]
    KLg = np.array([KL[s].sum() for s in g_slots])
    KHg = np.array([KH[s].sum() for s in g_slots])
    GW8 = int((KLg + KHg).max()) * 8

    idx_all = np.zeros((N_CORES, n_g, P, GW8), np.int16)
    cm_all = np.full((N_CORES, n_g, P, G_TILES * KT), 999.0, np.float32)
    wlo = wrap_rep(idx_lo.reshape(ntiles * P, KLmax)[None].reshape(
        ntiles, P, KLmax)) if False else wrap_rep(idx_lo)
    whi = wrap_rep(idx_hi)
    for c in range(N_CORES):
        for g, slots in enumerate(g_slots):
            off = 0
            for s in slots:                    # lo blocks
                jt = assign[c, s]
                w = int(KL[s]) * 8
                idx_all[c, g, :, off:off + w] = wlo[jt][:, :w]
                off += w
            for s in slots:                    # hi blocks
                jt = assign[c, s]
                w = int(KH[s]) * 8
                idx_all[c, g, :, off:off + w] = whi[jt][:, :w]
                off += w
            for si, s in enumerate(slots):
                jt = assign[c, s]
                cm_all[c, g, :, si * KT:(si + 1) * KT] = cm_tile[jt]

    x_t = np.zeros((IN_DIM, NP), BF16)
    x_t[:, :n] = np.asarray(x, np.float32).astype(BF16).T

    # iota_rep[p, t*KT + k] = t  (target-major one-hot comparand)
    iota_rep = np.repeat(np.arange(P, dtype=np.float32), KT)[None, :]
    iota_rep = np.broadcast_to(iota_rep, (P, P * KT)).astype(BF16)
    iota_rep = np.ascontiguousarray(iota_rep)

    # node-phase dinv columns [p, tile] per core
    dinv_ct = np.stack([dinv[c * nsh:(c + 1) * nsh].reshape(t_c, P).T
                        for c in range(N_CORES)])   # [cores, 128, T_C]
    # edge-phase dinv, in slot order per core: [cores, T_C*P] bf16
    dinv_tiles = dinv.reshape(ntiles, P)
    dinv_et = dinv_tiles[assign].reshape(N_CORES, t_c * P).astype(BF16)
    return dict(NP=NP, NSH=nsh, T_C=t_c, H0=h0, KL=KL, KH=KH, KT=KT,
                NG=n_g, G_SLOTS=g_slots, KLg=KLg, KHg=KHg, GW8=GW8,
                dinv_ct=dinv_ct, dinv_et=dinv_et, assign=assign,
                idx_all=idx_all, cm_all=cm_all.astype(BF16),
                x_t=x_t, iota_rep=iota_rep)


def _build_program(meta, with_collective=True):
    import concourse.bass as bass
    import concourse.bacc as bacc
    import concourse.mybir as mybir
    import concourse.tile as tile
    from concourse.masks import make_identity

    f32 = mybir.dt.float32
    bf16 = mybir.dt.bfloat16
    i16 = mybir.dt.int16
    NSH, T_C, NP, H0 = meta["NSH"], meta["T_C"], meta["NP"], meta["H0"]
    KL, KH, KT = meta["KL"], meta["KH"], meta["KT"]
    NG, G_SLOTS = meta["NG"], meta["G_SLOTS"]
    KLg, KHg, GW8 = meta["KLg"], meta["KHg"], meta["GW8"]
    AF = mybir.ActivationFunctionType
    Alu = mybir.AluOpType

    nc = bacc.Bacc("TRN2", target_bir_lowering=False, debug=False,
                   num_swdge_queues=4, dynamic_dma_scratch_size=65536)
    table = nc.dram_tensor("cc_table", [NP, HID], bf16, addr_space="Shared")

    # ---- external inputs (per-core shards unless noted)
    DSPL = ((T_C + 1) // 2) * P
    CWA = 3468 + 2 * T_C + (2 * T_C) % 2   # plane blob width (bf16 cols)
    CWR = 768 + T_C * P                    # partition-0 row blob width
    d_xt = nc.dram_tensor("x_t", [IN_DIM, NSH], bf16, kind="ExternalInput")
    d_cba = nc.dram_tensor("cba", [P, CWA], bf16, kind="ExternalInput")
    d_cbr = nc.dram_tensor("cbr", [1, CWR], bf16, kind="ExternalInput")
    d_idx = nc.dram_tensor("idx", [NG, P, GW8], i16, kind="ExternalInput")
    d_cm = nc.dram_tensor("cm", [NG, P, len(G_SLOTS[0]) * KT], bf16,
                          kind="ExternalInput")
    d_out = nc.dram_tensor("outp", [OUT_DIM, NSH], f32, kind="ExternalOutput")

    with tile.TileContext(nc) as tc:
        with (
            tc.tile_pool(name="const", bufs=1) as cpool,
            tc.tile_pool(name="sbuf", bufs=3) as pool,
            tc.tile_pool(name="dram", bufs=1, space="DRAM") as dpool,
        ):
            # ---------- persistent constants: one packed blob ----------
            cb = cpool.tile([P, CWA + CWR], bf16)
            nc.sync.dma_start(cb[:, :CWA], d_cba[:])
            nc.sync.dma_start(cb[0:1, CWA:], d_cbr[:])
            w_in_a = cb[:, 0:128]
            w_in_b = cb[:, 128:256]
            w_nor = cb[:, 256:384]          # zero-extended K=128 (host)
            w_ab = cb[:, 384:512]
            w_att = cb[:, 512:640]
            w_upd = cb[:, 640:768]
            w_cls = cb[:, 768:770]
            v_att = cb[:, 770:771]
            OIOTA = 772
            OIDENT = OIOTA + P * KT
            OF32 = OIDENT + P
            iota_rep = cb[:, OIOTA:OIDENT]
            ident = cb[:, OIDENT:OF32]
            b_nor = cb[:, OF32:OF32 + 2].bitcast(f32)
            b_ab = cb[:, OF32 + 2:OF32 + 4].bitcast(f32)
            b_att = cb[:, OF32 + 4:OF32 + 6].bitcast(f32)
            b_cls = cb[0:OUT_DIM, OF32 + 6:OF32 + 8].bitcast(f32)
            dct = cb[:, OF32 + 8:OF32 + 8 + 2 * T_C].bitcast(f32)
            b_in = cb[0:1, CWA:CWA + 128]       # rows: partition 0 only
            b_upd = cb[0:1, CWA + 128:CWA + 256]
            ones_r = cb[0:1, CWA + 256:CWA + 768]
            det_a = cb[0:1, CWA + 768:CWA + 768 + DSPL]
            det_b = cb[0:1, CWA + 768 + DSPL:CWA + CWR]

            # x shard in separate chunk tiles (tile-granular dep tracking:
            # one tile would make slab 0 wait for every chunk's DMA)
            XCH = SLAB_TILES * P * 4            # 2048 cols = 4 slabs
            xta_c, xtb_c = [], []
            for x0 in range(0, NSH, XCH):
                x1 = min(NSH, x0 + XCH)
                ta = cpool.tile([P, x1 - x0], bf16)
                tb = cpool.tile([P, x1 - x0], bf16)
                nc.sync.dma_start(ta[:], d_xt[:P, x0:x1])
                nc.sync.dma_start(tb[:], d_xt[P:, x0:x1])
                xta_c.append(ta)
                xtb_c.append(tb)

            # edge-phase dinv broadcast targets (broadcasts interleaved
            # into the edge loop so Pool isn't blocked before gather 0)
            dbc_a = cpool.tile([P, DSPL], bf16)
            dbc_b = cpool.tile([P, T_C * P - DSPL], bf16)

            # message table (gather source) + local shard
            shard = dpool.tile([NSH, HID], bf16)

            # prefetch ALL edge-phase index/cm data in two bulk DMAs
            # (independent of the table; overlaps the node phase)
            CMW = len(G_SLOTS[0]) * KT
            it_all = cpool.tile([P, NG * GW8], i16)
            nc.sync.dma_start(
                it_all[:].rearrange("p (g w) -> p g w", g=NG),
                d_idx[:].rearrange("g p w -> p g w"))
            cm_all = cpool.tile([P, NG * CMW], bf16)
            nc.sync.dma_start(
                cm_all[:].rearrange("p (g w) -> p g w", g=NG),
                d_cm[:].rearrange("g p w -> p g w"))

            # ---------- node phase (this core's NSH nodes) ----------
            with (
                tc.tile_pool(name="nsb", bufs=5) as npool,
                tc.tile_pool(name="npsum", bufs=2, space="PSUM") as pp1,
                tc.tile_pool(name="npnpa", bufs=1, space="PSUM") as ppn,
                tc.tile_pool(name="ntr", bufs=2, space="PSUM") as ppt,
            ):
                slabs = []
                t0 = 0
                while t0 < T_C:
                    nt = min(SLAB_TILES, T_C - t0)
                    slabs.append((t0, nt))
                    t0 += nt

                def node_head(s0, nt):
                    nn = nt * P
                    nb = s0 * P
                    # ph = W_in.T @ x + b_in (bias folded via ones-row matmul)
                    ck, co = nb // XCH, nb % XCH
                    ph = pp1.tile([P, 512], f32, tag="ph")
                    nc.tensor.matmul(ph[:, :nn], w_in_a[:],
                                     xta_c[ck][:, co:co + nn],
                                     start=True, stop=False)
                    nc.tensor.matmul(ph[:, :nn], w_in_b[:],
                                     xtb_c[ck][:, co:co + nn],
                                     start=False, stop=False)
                    nc.tensor.matmul(ph[:, :nn], b_in[:], ones_r[:, :nn],
                                     start=False, stop=True)
                    # leaky_relu(ph) = 0.01*ph + relu(0.99*ph)  (no Lrelu
                    # table: Relu shares the Tanh/Sigmoid act-func set)
                    r2 = npool.tile([P, 512], bf16, tag="r2")
                    nc.scalar.activation(r2[:, :nn], ph[:, :nn], AF.Relu,
                                         scale=1.0 - LRELU_SLOPE)
                    h = npool.tile([P, 512], bf16, tag="h")
                    nc.vector.scalar_tensor_tensor(
                        out=h[:, :nn], in0=ph[:, :nn], scalar=LRELU_SLOPE,
                        in1=r2[:, :nn], op0=Alu.mult, op1=Alu.add)
                    pnpa = ppn.tile([P, 1024], f32, tag="pnpa")
                    pn = pnpa[:, :512]
                    pa = pnpa[:, 512:]
                    nc.tensor.matmul(pn[:, :nn], w_nor[:], h[:, :nn],
                                     start=True, stop=True)
                    nc.tensor.matmul(pa[:, :nn], w_ab[:], h[:, :nn],
                                     start=True, stop=True)
                    xn = npool.tile([P, 512], bf16, tag="xn")
                    xa = npool.tile([P, 512], bf16, tag="xa")
                    nc.scalar.activation(xn[:, :nn], pn[:, :nn], AF.Identity,
                                         bias=b_nor[:])
                    nc.vector.tensor_scalar(out=xa[:, :nn], in0=pa[:, :nn],
                                            scalar1=b_ab[:, :1], scalar2=None,
                                            op0=Alu.add)
                    # patt = W_att.T @ (xn + xa): accumulate in PSUM
                    patt = pp1.tile([P, 512], f32, tag="patt")
                    nc.tensor.matmul(patt[:, :nn], w_att[:], xn[:, :nn],
                                     start=True, stop=False)
                    nc.tensor.matmul(patt[:, :nn], w_att[:], xa[:, :nn],
                                     start=False, stop=True)
                    hatt = npool.tile([P, 512], bf16, tag="hatt")
                    nc.scalar.activation(hatt[:, :nn], patt[:, :nn], AF.Tanh,
                                         bias=b_att[:])
                    return dict(s0=s0, nt=nt, xn=xn, xa=xa, hatt=hatt)

                def node_tail(st):
                    s0, nt, xn, xa, hatt = (st["s0"], st["nt"], st["xn"],
                                            st["xa"], st["hatt"])
                    nn = nt * P
                    nb = s0 * P
                    # alpha for the whole slab as a row: v_att.T @ hatt
                    pal = pp1.tile([1, 512], f32, tag="patt")
                    nc.tensor.matmul(pal[:, :nn], v_att[:], hatt[:, :nn],
                                     start=True, stop=True)
                    sig = npool.tile([1, 512], bf16, tag="sig")
                    nc.scalar.activation(sig[:, :nn], pal[:, :nn], AF.Sigmoid)
                    abc = npool.tile([P, 512], bf16, tag="abc")
                    nc.gpsimd.partition_broadcast(abc[:, :nn], sig[:, :nn])
                    # m_pre = xa + alpha*(xn - xa), feature-major
                    xd = npool.tile([P, 512], bf16, tag="xd")
                    nc.vector.tensor_sub(xd[:, :nn], xn[:, :nn], xa[:, :nn])
                    m1 = npool.tile([P, 512], bf16, tag="m1")
                    nc.vector.tensor_mul(m1[:, :nn], xd[:, :nn], abc[:, :nn])
                    m2t = npool.tile([P, 512], bf16, tag="m2t")
                    nc.vector.tensor_add(m2t[:, :nn], m1[:, :nn], xa[:, :nn])
                    # transpose to node-major; the dinv factor becomes a
                    # per-partition scalar applied during the PSUM->SBUF move
                    tp = ppt.tile([P, 512], bf16, tag="tr")
                    for j in range(nt):
                        nc.tensor.transpose(tp[:, j * P:(j + 1) * P],
                                            m2t[:, j * P:(j + 1) * P],
                                            ident[:])
                    mslab = npool.tile([P, SLAB_TILES * HID], bf16, tag="mslab")
                    for j in range(nt):
                        if j % 2 == 0:
                            nc.vector.tensor_scalar(
                                out=mslab[:, j * P:(j + 1) * P],
                                in0=tp[:, j * P:(j + 1) * P],
                                scalar1=dct[:, s0 + j:s0 + j + 1],
                                scalar2=None, op0=Alu.mult)
                        else:
                            nc.scalar.activation(
                                mslab[:, j * P:(j + 1) * P],
                                tp[:, j * P:(j + 1) * P], AF.Identity,
                                scale=dct[:, s0 + j:s0 + j + 1])
                    nc.sync.dma_start(
                        shard[nb:nb + nn, :]
                        .rearrange("(p j) f -> p (j f)", j=nt),
                        mslab[:, :nn])
                prev = None
                for si, (s0, nt) in enumerate(slabs):
                    st = node_head(s0, nt)
                    if prev is not None:
                        node_tail(prev)
                    if not with_collective and si == len(slabs) - 1:
                        # sim mode: first table half copied while the last
                        # slabs still compute (contiguous bulk copies)
                        hrow = (len(slabs) // 2 * SLAB_TILES) * P
                        nc.scalar.dma_start(table[:hrow, :], shard[:hrow, :])
                    prev = st
                node_tail(prev)
                if not with_collective:
                    hrow = (len(slabs) // 2 * SLAB_TILES) * P
                    nc.scalar.dma_start(table[hrow:NSH, :], shard[hrow:, :])

            # ---------- replicate table ----------
            if with_collective:
                nc.gpsimd.collective_compute(
                    "AllGather",
                    mybir.AluOpType.bypass,
                    replica_groups=[list(range(N_CORES))],
                    ins=[shard.opt()],
                    outs=[table[:]],
                )
            # (sim mode streams the local table copy inside the node loop)

            # ---------- edge phase (grouped gathers over target tiles) ----
            GWC = int((KLg + KHg).max())       # g-tile chunk capacity
            with (
                tc.tile_pool(name="esb", bufs=3) as ep,
                tc.tile_pool(name="soh", bufs=6) as sp,
                tc.tile_pool(name="epsum", bufs=4, space="PSUM") as pp2,
            ):
                def build_oh(g):
                    cb = g * CMW
                    s3s = []
                    for si in range(len(G_SLOTS[g])):
                        s_oh = sp.tile([P, P * KT], bf16, tag="soh")
                        s3 = s_oh[:].rearrange("p (t k) -> p t k", k=KT)
                        nc.vector.tensor_tensor(
                            out=s3,
                            in0=iota_rep[:].rearrange("p (t k) -> p t k", k=KT),
                            in1=cm_all[:, cb + si * KT:cb + (si + 1) * KT]
                            .unsqueeze(1).to_broadcast([P, P, KT]),
                            op=Alu.is_equal)
                        s3s.append(s3)
                    return s3s

                oh_cur = build_oh(0)
                for g in range(NG):
                    slots = G_SLOTS[g]
                    klg, khg = int(KLg[g]), int(KHg[g])
                    ib = g * GW8
                    gt = ep.tile([P, GWC * HID], bf16, tag="g")
                    nc.gpsimd.dma_gather(
                        out_ap=gt[:, :klg * HID]
                        .rearrange("p (c f) -> p c f", f=HID),
                        in_ap=table[0:H0, :],
                        idxs_ap=it_all[:, ib:ib + klg * 8],
                        num_idxs=klg * P,
                        num_idxs_reg=klg * P,
                        elem_size=HID,
                        queue_num=(2 * g) % 4,
                    )
                    nc.gpsimd.dma_gather(
                        out_ap=gt[:, klg * HID:(klg + khg) * HID]
                        .rearrange("p (c f) -> p c f", f=HID),
                        in_ap=table[H0:NP, :],
                        idxs_ap=it_all[:, ib + klg * 8:ib + (klg + khg) * 8],
                        num_idxs=khg * P,
                        num_idxs_reg=khg * P,
                        elem_size=HID,
                        queue_num=(2 * g + 1) % 4,
                    )
                    # dinv broadcasts ride behind the first two descgens
                    if g == 0:
                        nc.gpsimd.partition_broadcast(dbc_a[:], det_a[:])
                    elif g == 1:
                        nc.gpsimd.partition_broadcast(dbc_b[:], det_b[:])
                    # next group's one-hots ahead of this group's m2 chain
                    oh_next = build_oh(g + 1) if g + 1 < NG else None
                    klpre = khpre = 0
                    og = ep.tile([OUT_DIM, len(G_SLOTS[0]) * P], f32, tag="og")
                    for si, s in enumerate(slots):
                        kl, kh = int(KL[s]), int(KH[s])
                        s3 = oh_cur[si]
                        pagg = pp2.tile([P, P], f32, tag="pagg")
                        for c in range(kl):
                            nc.tensor.matmul(
                                pagg[:], gt[:, (klpre + c) * HID:
                                            (klpre + c + 1) * HID],
                                s3[:, :, c], start=(c == 0), stop=False)
                        for c in range(kh):
                            nc.tensor.matmul(
                                pagg[:], gt[:, (klg + khpre + c) * HID:
                                            (klg + khpre + c + 1) * HID],
                                s3[:, :, kl + c], start=False,
                                stop=(c == kh - 1))
                        klpre += kl
                        khpre += kh
                        jn = s * P
                        m2 = pool.tile([P, P], bf16, tag="m2")
                        dslc = (dbc_a[:, jn:jn + P] if jn + P <= DSPL
                                else dbc_b[:, jn - DSPL:jn - DSPL + P])
                        nc.vector.tensor_mul(m2[:], pagg[:], dslc)
                        pu = pp2.tile([P, P], f32, tag="emisc")
                        nc.tensor.matmul(pu[:], w_upd[:], m2[:],
                                         start=True, stop=False)
                        nc.tensor.matmul(pu[:], b_upd[:], ones_r[:, :P],
                                         start=False, stop=True)
                        r2e = pool.tile([P, P], bf16, tag="r2e")
                        nc.scalar.activation(r2e[:], pu[:], AF.Relu,
                                             scale=1.0 - LRELU_SLOPE)
                        lu = pool.tile([P, P], bf16, tag="lu")
                        nc.vector.scalar_tensor_tensor(
                            out=lu[:], in0=pu[:], scalar=LRELU_SLOPE,
                            in1=r2e[:], op0=Alu.mult, op1=Alu.add)
                        po = pp2.tile([OUT_DIM, P], f32, tag="emisc")
                        nc.tensor.matmul(po[:], w_cls[:], lu[:],
                                         start=True, stop=True)
                        nc.scalar.activation(og[:, si * P:(si + 1) * P],
                                             po[:], AF.Identity, bias=b_cls[:])
                    nc.scalar.dma_start(
                        d_out[:, slots[0] * P:(slots[-1] + 1) * P],
                        og[:, :len(slots) * P])
                    oh_cur = oh_next

    nc.compile()
    return nc


def _run_spmd_presharded(nc, in_maps, n_cores=N_CORES):
    """Run a compiled Bass program on n_cores via PJRT with host-side
    pre-sharded inputs (avoids XLA reshard programs on big arrays)."""
    import jax
    import concourse.mybir as mybir
    from concourse import bass2jax
    from jax.sharding import Mesh, PartitionSpec, NamedSharding
    from jax.experimental.shard_map import shard_map

    bass2jax.install_neuronx_cc_hook()
    partition_name = nc.partition_id_tensor.name if nc.partition_id_tensor else None
    in_names, out_names, out_avals, zero_outs = [], [], [], []
    for alloc in nc.m.functions[0].allocations:
        if not isinstance(alloc, mybir.MemoryLocationSet):
            continue
        name = alloc.memorylocations[0].name
        if alloc.kind == "ExternalInput":
            if name != partition_name:
                in_names.append(name)
        elif alloc.kind == "ExternalOutput":
            out_names.append(name)
            shape = tuple(alloc.tensor_shape)
            dtype = mybir.dt.np(alloc.dtype)
            out_avals.append(jax.core.ShapedArray(shape, dtype))
            zero_outs.append(np.zeros(shape, dtype))
    n_params = len(in_names)
    in_names_all = list(in_names) + out_names
    if partition_name is not None:
        in_names_all.append(partition_name)

    def _body(*args):
        operands = list(args)
        if partition_name is not None:
            operands.append(bass2jax.partition_id_tensor())
        outs = bass2jax._bass_exec_p.bind(
            *operands,
            out_avals=tuple(out_avals),
            in_names=tuple(in_names_all),
            out_names=tuple(out_names),
            lowering_input_output_aliases=(),
            sim_require_finite=True,
            sim_require_nnan=True,
            nc=nc,
        )
        return tuple(outs)

    devices = jax.devices()[:n_cores]
    mesh = Mesh(np.asarray(devices), ("core",))
    spec = PartitionSpec("core")
    n_outs = len(out_avals)
    sharded = jax.jit(
        shard_map(_body, mesh=mesh, in_specs=(spec,) * (n_params + n_outs),
                  out_specs=(spec,) * n_outs, check_rep=False),
        keep_unused=True,
    )
    sh = NamedSharding(mesh, spec)

    def put(per_core_arrays):
        a0 = np.asarray(per_core_arrays[0])
        gshape = (n_cores * a0.shape[0],) + a0.shape[1:]
        shards = [jax.device_put(np.ascontiguousarray(per_core_arrays[c]),
                                 devices[c]) for c in range(n_cores)]
        # block per input: too many in-flight transfers desync the axon mesh
        jax.block_until_ready(shards)
        return jax.make_array_from_single_device_arrays(gshape, sh, shards)

    args = [put([m[name] for m in in_maps]) for name in in_names]
    args += [put([z] * n_cores) for z in zero_outs]
    out_arrs = sharded(*args)
    jax.block_until_ready(out_arrs)
    return [
        {name: np.asarray(out_arrs[i]).reshape(n_cores, *out_avals[i].shape)[c]
         for i, name in enumerate(out_names)}
        for c in range(n_cores)
    ]


def kernel(x, edge_index, W_in, b_in, W_nor, b_nor, W_abnor, b_abnor,
           W_att, b_att, v_att, W_upd, b_upd, W_cls, b_cls):
    x = np.asarray(x, np.float32)
    n = x.shape[0]
    meta = _host_plan(x, edge_index)
    NSH, T_C = meta["NSH"], meta["T_C"]
    nc = _build_program(meta, with_collective=True)

    def bf(a):
        return np.asarray(a, np.float32).astype(BF16)

    T_C = meta["T_C"]
    KT = meta["KT"]
    DSPL = ((T_C + 1) // 2) * P
    OIOTA = 772
    OIDENT = OIOTA + P * KT
    OF32 = OIDENT + P
    CWA = OF32 + 8 + 2 * T_C
    CWR = 768 + T_C * P

    cba0 = np.zeros((P, CWA), BF16)
    wi = bf(W_in)
    cba0[:, 0:128] = wi[:P]
    cba0[:, 128:256] = wi[P:]
    cba0[:HALF, 256:384] = bf(W_nor)
    cba0[HALF:, 384:512] = bf(W_abnor)
    cba0[:, 512:640] = bf(W_att)
    cba0[:, 640:768] = bf(W_upd)
    cba0[:, 768:770] = bf(W_cls)
    cba0[:, 770:771] = bf(np.asarray(v_att).reshape(HID, 1))
    cba0[:, OIOTA:OIDENT] = meta["iota_rep"]
    cba0[:, OIDENT:OF32] = np.eye(P, dtype=np.float32).astype(BF16)
    cu = cba0.view(np.uint16)

    def put_f32(col, arr):
        a = np.ascontiguousarray(np.asarray(arr, np.float32))
        cu[:a.shape[0], col:col + 2 * a.shape[1]] = a.view(np.uint16)

    put_f32(OF32, np.asarray(b_nor, np.float32).reshape(HID, 1))
    put_f32(OF32 + 2, np.asarray(b_abnor, np.float32).reshape(HID, 1))
    put_f32(OF32 + 4, np.asarray(b_att, np.float32).reshape(HID, 1))
    put_f32(OF32 + 6, np.asarray(b_cls, np.float32).reshape(OUT_DIM, 1))

    cbr0 = np.zeros((1, CWR), BF16)
    cbr0[0, 0:128] = bf(np.asarray(b_in).reshape(HID))
    cbr0[0, 128:256] = bf(np.asarray(b_upd).reshape(HID))
    cbr0[0, 256:768] = BF16(1.0)

    in_maps = []
    for c in range(N_CORES):
        cba = cba0.copy()
        cba.view(np.uint16)[:, OF32 + 8:OF32 + 8 + 2 * T_C] = (
            np.ascontiguousarray(meta["dinv_ct"][c].astype(np.float32))
            .view(np.uint16))
        cbr = cbr0.copy()
        cbr[0, 768:768 + T_C * P] = meta["dinv_et"][c]
        in_maps.append({
            "cba": cba,
            "cbr": cbr,
            "x_t": np.ascontiguousarray(meta["x_t"][:, c * NSH:(c + 1) * NSH]),
            "idx": meta["idx_all"][c],
            "cm": meta["cm_all"][c],
        })

    results = None
    for attempt in range(3):
        try:
            results = _run_spmd_presharded(nc, in_maps)
            break
        except Exception:
            if attempt == 2:
                raise
            import time as _time
            _time.sleep(5.0)
    NPD = meta["NP"]
    out_full = np.empty((NPD, OUT_DIM), np.float32)
    assign = meta["assign"]
    for c in range(N_CORES):
        oc = results[c]["outp"].T.reshape(T_C, P, OUT_DIM)   # per slot
        for s_i in range(T_C):
            jt = assign[c, s_i]
            out_full[jt * P:(jt + 1) * P] = oc[s_i]
    return np.ascontiguousarray(out_full[:n])
